# revision 7
# baseline (speedup 1.0000x reference)
"""Trainium2 Bass kernel for ChannelCompression:
   y = minmax_norm_spatial(leaky_relu(circulant_1x1_conv(x) + b))

Sharding: pure data parallel over batch (16 batches -> 2 per core x 8 cores).

Per-core strategy (memory-roofline bound: read x once, write y once):
  - View each batch as [C=16, G=8, S=32768] and stack (c,g) onto the 128
    SBUF partitions.  The circulant 16x16 conv becomes one 128x128
    block-structured matmul weight kron(W2.T, I8), so every PE column
    computes all 16 output channels for 8 spatial groups at once.
  - Pass 1 streams x tiles in, matmuls into PSUM (full fp32), applies
    leaky-relu (+bias) on ScalarE while copying PSUM -> resident SBUF y
    buffer (16 MiB/batch), and reduces per-partition min/max on DVE.
  - Per-batch stats are folded across the 8 spatial groups via tiny PE
    transposes into free-dim space, reduced, inverted, and broadcast back
    to per-partition scale/bias with two tiny selector matmuls.
  - Pass 2 normalizes the resident y on GpSimd (one tensor_scalar per
    tile) and streams the result out.  Pass 2 of batch b is interleaved
    with pass 1 of batch b+1 to keep PE/DMA busy end to end.
"""

import numpy as np
from contextlib import ExitStack

import concourse.bacc as bacc
import concourse.tile as tile
import concourse.bass as bass
from concourse import mybir
from concourse.bass_utils import run_bass_kernel_spmd

F32 = mybir.dt.float32
AF = mybir.ActivationFunctionType
ALU = mybir.AluOpType
AX = mybir.AxisListType

N_CORES = 8
B, C, H, W = 16, 16, 512, 512
G = 8                   # spatial groups stacked into partitions
BP = B // N_CORES       # batches per core
S_FULL = (H * W) // G   # 32768 spatial elems per group
TS = 2048               # columns per resident y tile
PT = 1024               # columns per PSUM tile (2 banks)
MM = 512                # columns per matmul (1 PSUM bank, fp32 moving max)
EPS = 1e-8
NEG_SLOPE = 0.1


def build_nc(S=S_FULL, ts=TS):
    n_t = S // ts
    nc = bacc.Bacc("TRN2", target_bir_lowering=False)

    xs = nc.dram_tensor("x", [BP, C, G, S], F32, kind="ExternalInput")
    wbd = nc.dram_tensor("wbd", [128, 128], F32, kind="ExternalInput")
    ident = nc.dram_tensor("ident", [128, 128], F32, kind="ExternalInput")
    sel = nc.dram_tensor("sel", [32, 2, 128], F32, kind="ExternalInput")
    bb = nc.dram_tensor("b128", [128, 1], F32, kind="ExternalInput")
    ys = nc.dram_tensor("y", [BP, C, G, S], F32, kind="ExternalOutput")

    with tile.TileContext(nc) as tc, ExitStack() as ctx:
        consts = ctx.enter_context(tc.tile_pool(name="consts", bufs=1))
        xpool = ctx.enter_context(tc.tile_pool(name="xpool", bufs=2))
        ypool = ctx.enter_context(tc.tile_pool(name="ypool", bufs=n_t + 2))
        opool = ctx.enter_context(tc.tile_pool(name="opool", bufs=3))
        spool = ctx.enter_context(tc.tile_pool(name="stats", bufs=2))
        small = ctx.enter_context(tc.tile_pool(name="small", bufs=2))
        psum = ctx.enter_context(tc.tile_pool(name="psum", bufs=3, space="PSUM"))
        psmall = ctx.enter_context(tc.tile_pool(name="psmall", bufs=2, space="PSUM"))

        wbd_sb = consts.tile([128, 128], F32)
        nc.sync.dma_start(out=wbd_sb, in_=wbd[:])
        id_sb = consts.tile([128, 128], F32)
        nc.sync.dma_start(out=id_sb, in_=ident[:])
        sel_sb = consts.tile([32, 2, 128], F32)
        nc.sync.dma_start(out=sel_sb, in_=sel[:])
        b_sb = consts.tile([128, 1], F32)
        nc.sync.dma_start(out=b_sb, in_=bb[:])

        state = {}

        def pass1_tile(bi, i):
            """DMA in x tile i of batch bi, conv+lrelu into resident y, stats."""
            st_min, st_max, y_tiles = state[bi]
            xt = xpool.tile([128, ts], F32, tag="x")
            nc.sync.dma_start(out=xt, in_=xs[bi, :, :, i * ts:(i + 1) * ts])
            yt = ypool.tile([128, ts], F32, tag="y")
            for j in range(ts // PT):
                pt = psum.tile([128, PT], F32, tag="ps")
                for k in range(PT // MM):
                    c0 = k * MM
                    nc.tensor.matmul(
                        pt[:, c0:c0 + MM],
                        wbd_sb,
                        xt[:, j * PT + c0:j * PT + c0 + MM],
                        start=True,
                        stop=True,
                    )
                # y = leaky_relu(conv + b): fused PSUM->SBUF on ScalarE
                nc.scalar.activation(
                    out=yt[:, j * PT:(j + 1) * PT],
                    in_=pt,
                    func=AF.Prelu,
                    bias=b_sb,
                    scale=1.0,
                    alpha=NEG_SLOPE,
                )
            nc.vector.tensor_reduce(
                out=st_min[:, i:i + 1], in_=yt, axis=AX.X, op=ALU.min
            )
            nc.vector.tensor_reduce(
                out=st_max[:, i:i + 1], in_=yt, axis=AX.X, op=ALU.max
            )
            y_tiles.append(yt)

        def stats_fold(bi):
            """Fold per-partition stats into per-partition scale/bias [128,2]."""
            st_min, st_max, _ = state[bi]
            s2 = small.tile([128, 2], F32, tag="s2")
            nc.vector.tensor_reduce(out=s2[:, 0:1], in_=st_min, axis=AX.X, op=ALU.min)
            nc.vector.tensor_reduce(out=s2[:, 1:2], in_=st_max, axis=AX.X, op=ALU.max)
            # transpose [128,1] stats into free dim (partition 0)
            ptr_min = psmall.tile([1, 128], F32, tag="psm")
            nc.tensor.transpose(ptr_min, s2[:, 0:1], id_sb)
            ptr_max = psmall.tile([1, 128], F32, tag="psm")
            nc.tensor.transpose(ptr_max, s2[:, 1:2], id_sb)
            tl = small.tile([1, 256], F32, tag="tl")
            nc.scalar.copy(out=tl[:, 0:128], in_=ptr_min)
            nc.scalar.copy(out=tl[:, 128:256], in_=ptr_max)
            # reduce over the 8 groups (free index p = o*8+g)
            u = small.tile([1, 32], F32, tag="u")
            nc.vector.tensor_reduce(
                out=u[:, 0:16],
                in_=tl[:, 0:128].rearrange("p (o g) -> p o g", g=G),
                axis=AX.X,
                op=ALU.min,
            )
            nc.vector.tensor_reduce(
                out=u[:, 16:32],
                in_=tl[:, 128:256].rearrange("p (o g) -> p o g", g=G),
                axis=AX.X,
                op=ALU.max,
            )
            # scale = 1/(mx-mn+eps); nbias = -mn*scale
            v = small.tile([1, 16], F32, tag="v")
            nc.vector.tensor_sub(out=v, in0=u[:, 16:32], in1=u[:, 0:16])
            vv = small.tile([1, 16], F32, tag="vv")
            nc.vector.tensor_scalar(
                out=vv, in0=v, scalar1=EPS, scalar2=None, op0=ALU.add
            )
            pk = small.tile([1, 32], F32, tag="pk")
            nc.vector.reciprocal(out=pk[:, 0:16], in_=vv)
            tmp = small.tile([1, 16], F32, tag="tmp")
            nc.vector.tensor_mul(out=tmp, in0=u[:, 0:16], in1=pk[:, 0:16])
            nc.vector.tensor_scalar(
                out=pk[:, 16:32], in0=tmp, scalar1=-1.0, scalar2=None, op0=ALU.mult
            )
            # broadcast [1,32] free-dim -> per-partition [128,2] via transpose
            # + selector matmuls (sel[k,0,p]=d(k==p//8), sel[k,1,p]=d(k-16==p//8))
            pz = psmall.tile([32, 1], F32, tag="psm")
            nc.tensor.transpose(pz, pk, id_sb[0:1, 0:1])
            zs = small.tile([32, 1], F32, tag="zs")
            nc.scalar.copy(out=zs, in_=pz)
            pb1 = psmall.tile([128, 1], F32, tag="psm")
            nc.tensor.matmul(pb1, sel_sb[:, 0, :], zs, start=True, stop=True)
            pb2 = psmall.tile([128, 1], F32, tag="psm")
            nc.tensor.matmul(pb2, sel_sb[:, 1, :], zs, start=True, stop=True)
            sc = small.tile([128, 2], F32, tag="sc")
            nc.scalar.copy(out=sc[:, 0:1], in_=pb1)
            nc.scalar.copy(out=sc[:, 1:2], in_=pb2)
            return sc

        def pass2_tile(bi, i, sc):
            """Normalize resident y tile (GpSimd) and stream out."""
            _, _, y_tiles = state[bi]
            ot = opool.tile([128, ts], F32, tag="o")
            nc.gpsimd.tensor_scalar(
                out=ot,
                in0=y_tiles[i],
                scalar1=sc[:, 0:1],
                scalar2=sc[:, 1:2],
                op0=ALU.mult,
                op1=ALU.add,
            )
            nc.sync.dma_start(out=ys[bi, :, :, i * ts:(i + 1) * ts], in_=ot)

        # software pipeline: pass1(0); then per batch: pre-emit the first
        # next-batch tiles (keeps PE busy through the stats fold), fold
        # stats, then interleave pass2(bi) with the rest of pass1(bi+1).
        PRE = 2  # spare ypool slots usable before pass2 frees more
        for bi in range(BP):
            state[bi] = (
                spool.tile([128, n_t], F32, tag="stmin", name=f"stmin{bi}"),
                spool.tile([128, n_t], F32, tag="stmax", name=f"stmax{bi}"),
                [],
            )
        for i in range(n_t):
            pass1_tile(0, i)
        for bi in range(BP):
            if bi + 1 < BP:
                for i in range(PRE):
                    pass1_tile(bi + 1, i)
            sc = stats_fold(bi)
            for i in range(n_t):
                pass2_tile(bi, i, sc)
                if bi + 1 < BP and i + PRE < n_t:
                    pass1_tile(bi + 1, i + PRE)

    nc.compile()
    return nc


def host_consts(w, b):
    """Host-side tiny constant tensors fed to every core."""
    w = np.asarray(w, np.float32).reshape(16)
    b = np.asarray(b, np.float32).reshape(1)
    W2 = np.stack([np.roll(w, o) for o in range(16)], axis=0)   # [O,C]
    wbd = np.kron(W2.T.copy(), np.eye(G, dtype=np.float32))     # [128,128]
    wbd = np.ascontiguousarray(wbd, np.float32)
    ident = np.eye(128, dtype=np.float32)
    sel = np.zeros((32, 2, 128), np.float32)
    for p in range(128):
        sel[p // G, 0, p] = 1.0
        sel[16 + p // G, 1, p] = 1.0
    b128 = np.full((128, 1), float(b[0]), np.float32)
    return wbd, ident, sel, b128


_NC = None
LAST_RESULTS = None


def kernel(x, w, b):
    global _NC, LAST_RESULTS
    x = np.ascontiguousarray(np.asarray(x, np.float32))
    assert x.shape == (B, C, H, W)
    if _NC is None:
        _NC = build_nc()
    wbd, ident, sel, b128 = host_consts(w, b)

    xg = x.reshape(N_CORES, BP, C, G, S_FULL)
    in_maps = [
        {
            "x": np.ascontiguousarray(xg[ci]),
            "wbd": wbd,
            "ident": ident,
            "sel": sel,
            "b128": b128,
        }
        for ci in range(N_CORES)
    ]
    res = run_bass_kernel_spmd(_NC, in_maps, core_ids=list(range(N_CORES)))
    LAST_RESULTS = res
    out = np.concatenate([r["y"].reshape(BP, C, H, W) for r in res.results], axis=0)
    return out


# revision 8
# speedup vs baseline: 1.0253x; 1.0253x over previous
"""Trainium2 Bass kernel for ChannelCompression:
   y = minmax_norm_spatial(leaky_relu(circulant_1x1_conv(x) + b))

Sharding: pure data parallel over batch (16 batches -> 2 per core x 8 cores).

Per-core strategy (memory-roofline bound: read x once, write y once):
  - View each batch as [C=16, G=8, S=32768] and stack (c,g) onto the 128
    SBUF partitions.  The circulant 16x16 conv becomes one 128x128
    block-structured matmul weight kron(W2.T, I8), so every PE column
    computes all 16 output channels for 8 spatial groups at once.
  - Pass 1 streams x tiles in, matmuls into PSUM (full fp32), applies
    leaky-relu (+bias) on ScalarE while copying PSUM -> resident SBUF y
    buffer (16 MiB/batch), and reduces per-partition min/max on DVE.
  - Per-batch stats are folded across the 8 spatial groups via tiny PE
    transposes into free-dim space, reduced, inverted, and broadcast back
    to per-partition scale/bias with two tiny selector matmuls.
  - Pass 2 normalizes the resident y on GpSimd (one tensor_scalar per
    tile) and streams the result out.  Pass 2 of batch b is interleaved
    with pass 1 of batch b+1 to keep PE/DMA busy end to end.
"""

import numpy as np
from contextlib import ExitStack

import concourse.bacc as bacc
import concourse.tile as tile
import concourse.bass as bass
from concourse import mybir
from concourse.bass_utils import run_bass_kernel_spmd

F32 = mybir.dt.float32
AF = mybir.ActivationFunctionType
ALU = mybir.AluOpType
AX = mybir.AxisListType

N_CORES = 8
B, C, H, W = 16, 16, 512, 512
G = 8                   # spatial groups stacked into partitions
BP = B // N_CORES       # batches per core
S_FULL = (H * W) // G   # 32768 spatial elems per group
TS = 2048               # columns per resident y tile
PT = 1024               # columns per PSUM tile (2 banks)
MM = 512                # columns per matmul (1 PSUM bank, fp32 moving max)
EPS = 1e-8
NEG_SLOPE = 0.1


def build_nc(S=S_FULL, ts=TS):
    n_t = S // ts
    nc = bacc.Bacc("TRN2", target_bir_lowering=False)

    xs = nc.dram_tensor("x", [BP, C, G, S], F32, kind="ExternalInput")
    wbd = nc.dram_tensor("wbd", [128, 128], F32, kind="ExternalInput")
    ident = nc.dram_tensor("ident", [128, 128], F32, kind="ExternalInput")
    sel = nc.dram_tensor("sel", [32, 2, 128], F32, kind="ExternalInput")
    bb = nc.dram_tensor("b128", [128, 1], F32, kind="ExternalInput")
    ys = nc.dram_tensor("y", [BP, C, G, S], F32, kind="ExternalOutput")

    with tile.TileContext(nc) as tc, ExitStack() as ctx:
        consts = ctx.enter_context(tc.tile_pool(name="consts", bufs=1))
        xpool = ctx.enter_context(tc.tile_pool(name="xpool", bufs=2))
        ypool = ctx.enter_context(tc.tile_pool(name="ypool", bufs=n_t + 2))
        opool = ctx.enter_context(tc.tile_pool(name="opool", bufs=3))
        spool = ctx.enter_context(tc.tile_pool(name="stats", bufs=2))
        small = ctx.enter_context(tc.tile_pool(name="small", bufs=2))
        psum = ctx.enter_context(tc.tile_pool(name="psum", bufs=3, space="PSUM"))
        psmall = ctx.enter_context(tc.tile_pool(name="psmall", bufs=2, space="PSUM"))

        wbd_sb = consts.tile([128, 128], F32)
        nc.sync.dma_start(out=wbd_sb, in_=wbd[:])
        id_sb = consts.tile([128, 128], F32)
        nc.sync.dma_start(out=id_sb, in_=ident[:])
        sel_sb = consts.tile([32, 2, 128], F32)
        nc.sync.dma_start(out=sel_sb, in_=sel[:])
        b_sb = consts.tile([128, 1], F32)
        nc.sync.dma_start(out=b_sb, in_=bb[:])

        state = {}

        def pass1_tile(bi, i):
            """DMA in x tile i of batch bi, conv+lrelu into resident y, stats."""
            st_min, st_max, y_tiles = state[bi]
            xt = xpool.tile([128, ts], F32, tag="x")
            nc.sync.dma_start(out=xt, in_=xs[bi, :, :, i * ts:(i + 1) * ts])
            yt = ypool.tile([128, ts], F32, tag="y")
            for j in range(ts // PT):
                pt = psum.tile([128, PT], F32, tag="ps")
                for k in range(PT // MM):
                    c0 = k * MM
                    nc.tensor.matmul(
                        pt[:, c0:c0 + MM],
                        wbd_sb,
                        xt[:, j * PT + c0:j * PT + c0 + MM],
                        start=True,
                        stop=True,
                    )
                # y = leaky_relu(conv + b): fused PSUM->SBUF on ScalarE
                nc.scalar.activation(
                    out=yt[:, j * PT:(j + 1) * PT],
                    in_=pt,
                    func=AF.Prelu,
                    bias=b_sb,
                    scale=1.0,
                    alpha=NEG_SLOPE,
                )
            nc.vector.tensor_reduce(
                out=st_min[:, i:i + 1], in_=yt, axis=AX.X, op=ALU.min
            )
            nc.vector.tensor_reduce(
                out=st_max[:, i:i + 1], in_=yt, axis=AX.X, op=ALU.max
            )
            y_tiles.append(yt)

        def stats_fold(bi):
            """Fold per-partition stats into per-partition scale/bias [128,2]."""
            st_min, st_max, _ = state[bi]
            s2 = small.tile([128, 2], F32, tag="s2")
            nc.vector.tensor_reduce(out=s2[:, 0:1], in_=st_min, axis=AX.X, op=ALU.min)
            nc.vector.tensor_reduce(out=s2[:, 1:2], in_=st_max, axis=AX.X, op=ALU.max)
            # transpose [128,1] stats into free dim (partition 0)
            ptr_min = psmall.tile([1, 128], F32, tag="psm")
            nc.tensor.transpose(ptr_min, s2[:, 0:1], id_sb)
            ptr_max = psmall.tile([1, 128], F32, tag="psm")
            nc.tensor.transpose(ptr_max, s2[:, 1:2], id_sb)
            tl = small.tile([1, 256], F32, tag="tl")
            nc.scalar.copy(out=tl[:, 0:128], in_=ptr_min)
            nc.scalar.copy(out=tl[:, 128:256], in_=ptr_max)
            # reduce over the 8 groups (free index p = o*8+g)
            u = small.tile([1, 32], F32, tag="u")
            nc.vector.tensor_reduce(
                out=u[:, 0:16],
                in_=tl[:, 0:128].rearrange("p (o g) -> p o g", g=G),
                axis=AX.X,
                op=ALU.min,
            )
            nc.vector.tensor_reduce(
                out=u[:, 16:32],
                in_=tl[:, 128:256].rearrange("p (o g) -> p o g", g=G),
                axis=AX.X,
                op=ALU.max,
            )
            # scale = 1/(mx-mn+eps); nbias = -mn*scale
            v = small.tile([1, 16], F32, tag="v")
            nc.vector.tensor_sub(out=v, in0=u[:, 16:32], in1=u[:, 0:16])
            vv = small.tile([1, 16], F32, tag="vv")
            nc.vector.tensor_scalar(
                out=vv, in0=v, scalar1=EPS, scalar2=None, op0=ALU.add
            )
            pk = small.tile([1, 32], F32, tag="pk")
            nc.vector.reciprocal(out=pk[:, 0:16], in_=vv)
            tmp = small.tile([1, 16], F32, tag="tmp")
            nc.vector.tensor_mul(out=tmp, in0=u[:, 0:16], in1=pk[:, 0:16])
            nc.vector.tensor_scalar(
                out=pk[:, 16:32], in0=tmp, scalar1=-1.0, scalar2=None, op0=ALU.mult
            )
            # broadcast [1,32] free-dim -> per-partition [128,2] via transpose
            # + selector matmuls (sel[k,0,p]=d(k==p//8), sel[k,1,p]=d(k-16==p//8))
            pz = psmall.tile([32, 1], F32, tag="psm")
            nc.tensor.transpose(pz, pk, id_sb[0:1, 0:1])
            zs = small.tile([32, 1], F32, tag="zs")
            nc.scalar.copy(out=zs, in_=pz)
            pb1 = psmall.tile([128, 1], F32, tag="psm")
            nc.tensor.matmul(pb1, sel_sb[:, 0, :], zs, start=True, stop=True)
            pb2 = psmall.tile([128, 1], F32, tag="psm")
            nc.tensor.matmul(pb2, sel_sb[:, 1, :], zs, start=True, stop=True)
            sc = small.tile([128, 2], F32, tag="sc")
            nc.scalar.copy(out=sc[:, 0:1], in_=pb1)
            nc.scalar.copy(out=sc[:, 1:2], in_=pb2)
            return sc

        def pass2_tile(bi, i, sc):
            """Normalize resident y tile (GpSimd) and stream out."""
            _, _, y_tiles = state[bi]
            ot = opool.tile([128, ts], F32, tag="o")
            nc.gpsimd.tensor_scalar(
                out=ot,
                in0=y_tiles[i],
                scalar1=sc[:, 0:1],
                scalar2=sc[:, 1:2],
                op0=ALU.mult,
                op1=ALU.add,
            )
            nc.sync.dma_start(out=ys[bi, :, :, i * ts:(i + 1) * ts], in_=ot)

        # software pipeline: pass1(0); then per batch: pre-emit the first
        # next-batch tiles (keeps PE busy through the stats fold), fold
        # stats, then interleave pass2(bi) with the rest of pass1(bi+1).
        PRE = 0  # spare ypool slots usable before pass2 frees more
        for bi in range(BP):
            state[bi] = (
                spool.tile([128, n_t], F32, tag="stmin", name=f"stmin{bi}"),
                spool.tile([128, n_t], F32, tag="stmax", name=f"stmax{bi}"),
                [],
            )
        for i in range(n_t):
            pass1_tile(0, i)
        for bi in range(BP):
            if bi + 1 < BP:
                for i in range(PRE):
                    pass1_tile(bi + 1, i)
            sc = stats_fold(bi)
            for i in range(n_t):
                pass2_tile(bi, i, sc)
                if bi + 1 < BP and i + PRE < n_t:
                    pass1_tile(bi + 1, i + PRE)

    nc.compile()
    return nc


def host_consts(w, b):
    """Host-side tiny constant tensors fed to every core."""
    w = np.asarray(w, np.float32).reshape(16)
    b = np.asarray(b, np.float32).reshape(1)
    W2 = np.stack([np.roll(w, o) for o in range(16)], axis=0)   # [O,C]
    wbd = np.kron(W2.T.copy(), np.eye(G, dtype=np.float32))     # [128,128]
    wbd = np.ascontiguousarray(wbd, np.float32)
    ident = np.eye(128, dtype=np.float32)
    sel = np.zeros((32, 2, 128), np.float32)
    for p in range(128):
        sel[p // G, 0, p] = 1.0
        sel[16 + p // G, 1, p] = 1.0
    b128 = np.full((128, 1), float(b[0]), np.float32)
    return wbd, ident, sel, b128


_NC = None
LAST_RESULTS = None


def kernel(x, w, b):
    global _NC, LAST_RESULTS
    x = np.ascontiguousarray(np.asarray(x, np.float32))
    assert x.shape == (B, C, H, W)
    if _NC is None:
        _NC = build_nc()
    wbd, ident, sel, b128 = host_consts(w, b)

    xg = x.reshape(N_CORES, BP, C, G, S_FULL)
    in_maps = [
        {
            "x": np.ascontiguousarray(xg[ci]),
            "wbd": wbd,
            "ident": ident,
            "sel": sel,
            "b128": b128,
        }
        for ci in range(N_CORES)
    ]
    res = run_bass_kernel_spmd(_NC, in_maps, core_ids=list(range(N_CORES)))
    LAST_RESULTS = res
    out = np.concatenate([r["y"].reshape(BP, C, H, W) for r in res.results], axis=0)
    return out


# revision 9
# speedup vs baseline: 1.1254x; 1.0976x over previous
"""Trainium2 Bass kernel for ChannelCompression:
   y = minmax_norm_spatial(leaky_relu(circulant_1x1_conv(x) + b))

Sharding: pure data parallel over batch (16 batches -> 2 per core x 8 cores).

Per-core strategy (memory-roofline bound: read x once, write y once):
  - View each batch as [C=16, G=8, S=32768] and stack (c,g) onto the 128
    SBUF partitions.  The circulant 16x16 conv becomes one 128x128
    block-structured matmul weight kron(W2.T, I8), so every PE column
    computes all 16 output channels for 8 spatial groups at once.
  - Pass 1 streams x tiles in, matmuls into PSUM (full fp32), applies
    leaky-relu (+bias) on ScalarE while copying PSUM -> resident SBUF y
    buffer (16 MiB/batch), and reduces per-partition min/max on DVE.
  - Per-batch stats are folded across the 8 spatial groups via tiny PE
    transposes into free-dim space, reduced, inverted, and broadcast back
    to per-partition scale/bias with two tiny selector matmuls.
  - Pass 2 normalizes the resident y on GpSimd (one tensor_scalar per
    tile) and streams the result out.  Pass 2 of batch b is interleaved
    with pass 1 of batch b+1 to keep PE/DMA busy end to end.
"""

import numpy as np
from contextlib import ExitStack

import concourse.bacc as bacc
import concourse.tile as tile
import concourse.bass as bass
from concourse import mybir
from concourse.bass_utils import run_bass_kernel_spmd

F32 = mybir.dt.float32
AF = mybir.ActivationFunctionType
ALU = mybir.AluOpType
AX = mybir.AxisListType

N_CORES = 8
B, C, H, W = 16, 16, 512, 512
G = 8                   # spatial groups stacked into partitions
BP = B // N_CORES       # batches per core
S_FULL = (H * W) // G   # 32768 spatial elems per group
TS = 2048               # columns per resident y tile
PT = 1024               # columns per PSUM tile (2 banks)
MM = 512                # columns per matmul (1 PSUM bank, fp32 moving max)
EPS = 1e-8
NEG_SLOPE = 0.1


def build_nc(S=S_FULL, ts=TS):
    n_t = S // ts
    nc = bacc.Bacc("TRN2", target_bir_lowering=False)

    xs = nc.dram_tensor("x", [BP, C, G, S], F32, kind="ExternalInput")
    wbd = nc.dram_tensor("wbd", [128, 128], F32, kind="ExternalInput")
    ident = nc.dram_tensor("ident", [128, 128], F32, kind="ExternalInput")
    sel = nc.dram_tensor("sel", [32, 2, 128], F32, kind="ExternalInput")
    bb = nc.dram_tensor("b128", [128, 1], F32, kind="ExternalInput")
    ys = nc.dram_tensor("y", [BP, C, G, S], F32, kind="ExternalOutput")

    with tile.TileContext(nc) as tc, ExitStack() as ctx:
        consts = ctx.enter_context(tc.tile_pool(name="consts", bufs=1))
        xpool = ctx.enter_context(tc.tile_pool(name="xpool", bufs=3))
        ypool = ctx.enter_context(tc.tile_pool(name="ypool", bufs=n_t + 2))
        opool = ctx.enter_context(tc.tile_pool(name="opool", bufs=2))
        spool = ctx.enter_context(tc.tile_pool(name="stats", bufs=2))
        small = ctx.enter_context(tc.tile_pool(name="small", bufs=2))
        psum = ctx.enter_context(tc.tile_pool(name="psum", bufs=3, space="PSUM"))
        psmall = ctx.enter_context(tc.tile_pool(name="psmall", bufs=2, space="PSUM"))

        wbd_sb = consts.tile([128, 128], F32)
        nc.sync.dma_start(out=wbd_sb, in_=wbd[:])
        id_sb = consts.tile([128, 128], F32)
        nc.sync.dma_start(out=id_sb, in_=ident[:])
        sel_sb = consts.tile([32, 2, 128], F32)
        nc.sync.dma_start(out=sel_sb, in_=sel[:])
        b_sb = consts.tile([128, 1], F32)
        nc.sync.dma_start(out=b_sb, in_=bb[:])

        state = {}

        def pass1_tile(bi, i):
            """DMA in x tile i of batch bi, conv+lrelu into resident y, stats."""
            st_min, st_max, y_tiles = state[bi]
            xt = xpool.tile([128, ts], F32, tag="x")
            nc.sync.dma_start(out=xt, in_=xs[bi, :, :, i * ts:(i + 1) * ts])
            yt = ypool.tile([128, ts], F32, tag="y")
            for j in range(ts // PT):
                pt = psum.tile([128, PT], F32, tag="ps")
                for k in range(PT // MM):
                    c0 = k * MM
                    nc.tensor.matmul(
                        pt[:, c0:c0 + MM],
                        wbd_sb,
                        xt[:, j * PT + c0:j * PT + c0 + MM],
                        start=True,
                        stop=True,
                    )
                # y = leaky_relu(conv + b): fused PSUM->SBUF on ScalarE
                nc.scalar.activation(
                    out=yt[:, j * PT:(j + 1) * PT],
                    in_=pt,
                    func=AF.Prelu,
                    bias=b_sb,
                    scale=1.0,
                    alpha=NEG_SLOPE,
                )
            nc.vector.tensor_reduce(
                out=st_min[:, i:i + 1], in_=yt, axis=AX.X, op=ALU.min
            )
            nc.vector.tensor_reduce(
                out=st_max[:, i:i + 1], in_=yt, axis=AX.X, op=ALU.max
            )
            y_tiles.append(yt)

        def stats_fold(bi):
            """Fold per-partition stats into per-partition scale/bias [128,2]."""
            st_min, st_max, _ = state[bi]
            s2 = small.tile([128, 2], F32, tag="s2")
            nc.vector.tensor_reduce(out=s2[:, 0:1], in_=st_min, axis=AX.X, op=ALU.min)
            nc.vector.tensor_reduce(out=s2[:, 1:2], in_=st_max, axis=AX.X, op=ALU.max)
            # transpose [128,1] stats into free dim (partition 0)
            ptr_min = psmall.tile([1, 128], F32, tag="psm")
            nc.tensor.transpose(ptr_min, s2[:, 0:1], id_sb)
            ptr_max = psmall.tile([1, 128], F32, tag="psm")
            nc.tensor.transpose(ptr_max, s2[:, 1:2], id_sb)
            tl = small.tile([1, 256], F32, tag="tl")
            nc.scalar.copy(out=tl[:, 0:128], in_=ptr_min)
            nc.scalar.copy(out=tl[:, 128:256], in_=ptr_max)
            # reduce over the 8 groups (free index p = o*8+g)
            u = small.tile([1, 32], F32, tag="u")
            nc.vector.tensor_reduce(
                out=u[:, 0:16],
                in_=tl[:, 0:128].rearrange("p (o g) -> p o g", g=G),
                axis=AX.X,
                op=ALU.min,
            )
            nc.vector.tensor_reduce(
                out=u[:, 16:32],
                in_=tl[:, 128:256].rearrange("p (o g) -> p o g", g=G),
                axis=AX.X,
                op=ALU.max,
            )
            # scale = 1/(mx-mn+eps); nbias = -mn*scale
            v = small.tile([1, 16], F32, tag="v")
            nc.vector.tensor_sub(out=v, in0=u[:, 16:32], in1=u[:, 0:16])
            vv = small.tile([1, 16], F32, tag="vv")
            nc.vector.tensor_scalar(
                out=vv, in0=v, scalar1=EPS, scalar2=None, op0=ALU.add
            )
            pk = small.tile([1, 32], F32, tag="pk")
            nc.vector.reciprocal(out=pk[:, 0:16], in_=vv)
            tmp = small.tile([1, 16], F32, tag="tmp")
            nc.vector.tensor_mul(out=tmp, in0=u[:, 0:16], in1=pk[:, 0:16])
            nc.vector.tensor_scalar(
                out=pk[:, 16:32], in0=tmp, scalar1=-1.0, scalar2=None, op0=ALU.mult
            )
            # broadcast [1,32] free-dim -> per-partition [128,2] via transpose
            # + selector matmuls (sel[k,0,p]=d(k==p//8), sel[k,1,p]=d(k-16==p//8))
            pz = psmall.tile([32, 1], F32, tag="psm")
            nc.tensor.transpose(pz, pk, id_sb[0:1, 0:1])
            zs = small.tile([32, 1], F32, tag="zs")
            nc.scalar.copy(out=zs, in_=pz)
            pb1 = psmall.tile([128, 1], F32, tag="psm")
            nc.tensor.matmul(pb1, sel_sb[:, 0, :], zs, start=True, stop=True)
            pb2 = psmall.tile([128, 1], F32, tag="psm")
            nc.tensor.matmul(pb2, sel_sb[:, 1, :], zs, start=True, stop=True)
            sc = small.tile([128, 2], F32, tag="sc")
            nc.scalar.copy(out=sc[:, 0:1], in_=pb1)
            nc.scalar.copy(out=sc[:, 1:2], in_=pb2)
            return sc

        def pass2_tile(bi, i, sc):
            """Normalize resident y tile (GpSimd) and stream out."""
            _, _, y_tiles = state[bi]
            ot = opool.tile([128, ts], F32, tag="o")
            nc.gpsimd.tensor_scalar(
                out=ot,
                in0=y_tiles[i],
                scalar1=sc[:, 0:1],
                scalar2=sc[:, 1:2],
                op0=ALU.mult,
                op1=ALU.add,
            )
            nc.sync.dma_start(out=ys[bi, :, :, i * ts:(i + 1) * ts], in_=ot)

        # software pipeline: pass1(0); then per batch: pre-emit the first
        # next-batch tiles (keeps PE busy through the stats fold), fold
        # stats, then interleave pass2(bi) with the rest of pass1(bi+1).
        PRE = 0  # spare ypool slots usable before pass2 frees more
        for bi in range(BP):
            state[bi] = (
                spool.tile([128, n_t], F32, tag="stmin", name=f"stmin{bi}"),
                spool.tile([128, n_t], F32, tag="stmax", name=f"stmax{bi}"),
                [],
            )
        for i in range(n_t):
            pass1_tile(0, i)
        for bi in range(BP):
            if bi + 1 < BP:
                for i in range(PRE):
                    pass1_tile(bi + 1, i)
            sc = stats_fold(bi)
            for i in range(n_t):
                pass2_tile(bi, i, sc)
                if bi + 1 < BP and i + PRE < n_t:
                    pass1_tile(bi + 1, i + PRE)

    nc.compile()
    return nc


def host_consts(w, b):
    """Host-side tiny constant tensors fed to every core."""
    w = np.asarray(w, np.float32).reshape(16)
    b = np.asarray(b, np.float32).reshape(1)
    W2 = np.stack([np.roll(w, o) for o in range(16)], axis=0)   # [O,C]
    wbd = np.kron(W2.T.copy(), np.eye(G, dtype=np.float32))     # [128,128]
    wbd = np.ascontiguousarray(wbd, np.float32)
    ident = np.eye(128, dtype=np.float32)
    sel = np.zeros((32, 2, 128), np.float32)
    for p in range(128):
        sel[p // G, 0, p] = 1.0
        sel[16 + p // G, 1, p] = 1.0
    b128 = np.full((128, 1), float(b[0]), np.float32)
    return wbd, ident, sel, b128


_NC = None
LAST_RESULTS = None


def kernel(x, w, b):
    global _NC, LAST_RESULTS
    x = np.ascontiguousarray(np.asarray(x, np.float32))
    assert x.shape == (B, C, H, W)
    if _NC is None:
        _NC = build_nc()
    wbd, ident, sel, b128 = host_consts(w, b)

    xg = x.reshape(N_CORES, BP, C, G, S_FULL)
    in_maps = [
        {
            "x": np.ascontiguousarray(xg[ci]),
            "wbd": wbd,
            "ident": ident,
            "sel": sel,
            "b128": b128,
        }
        for ci in range(N_CORES)
    ]
    res = run_bass_kernel_spmd(_NC, in_maps, core_ids=list(range(N_CORES)))
    LAST_RESULTS = res
    out = np.concatenate([r["y"].reshape(BP, C, H, W) for r in res.results], axis=0)
    return out


# revision 30
# speedup vs baseline: 1.1682x; 1.0381x over previous
"""Trainium2 Bass kernel for ChannelCompression:
   y = minmax_norm_spatial(leaky_relu(circulant_1x1_conv(x) + b))

Sharding: pure data parallel over batch (16 batches -> 2 per core x 8 cores).

Per-core strategy (memory-roofline bound: read x once, write y once):
  - View each batch as [C=16, G=8, S=32768] and stack (c,g) onto the 128
    SBUF partitions.  The circulant 16x16 conv becomes one 128x128
    block-structured matmul weight kron(W2.T, I8), so every PE column
    computes all 16 output channels for 8 spatial groups at once.
  - Pass 1 streams x tiles in, matmuls into PSUM (full fp32), applies
    leaky-relu (+bias) on ScalarE while copying PSUM -> resident SBUF y
    buffer (16 MiB/batch), and reduces per-partition min/max on DVE.
  - Per-batch stats are folded across the 8 spatial groups via tiny PE
    transposes into free-dim space, reduced, inverted, and broadcast back
    to per-partition scale/bias with two tiny selector matmuls.
  - Pass 2 normalizes the resident y on GpSimd (one tensor_scalar per
    tile) and streams the result out.  Pass 2 of batch b is interleaved
    with pass 1 of batch b+1 to keep PE/DMA busy end to end.
"""

import numpy as np
from contextlib import ExitStack

import concourse.bacc as bacc
import concourse.tile as tile
import concourse.bass as bass
from concourse import mybir
from concourse.bass_utils import run_bass_kernel_spmd

F32 = mybir.dt.float32
AF = mybir.ActivationFunctionType
ALU = mybir.AluOpType
AX = mybir.AxisListType

N_CORES = 8
B, C, H, W = 16, 16, 512, 512
G = 8                   # spatial groups stacked into partitions
BP = B // N_CORES       # batches per core
S_FULL = (H * W) // G   # 32768 spatial elems per group
TS = 2048               # columns per resident y tile
PT = 1024               # columns per PSUM tile (2 banks)
MM = 512                # columns per matmul (1 PSUM bank, fp32 moving max)
EPS = 1e-8
NEG_SLOPE = 0.1


def build_nc(S=S_FULL, ts=TS):
    n_t = S // ts
    nc = bacc.Bacc("TRN2", target_bir_lowering=False)

    xs = nc.dram_tensor("x", [BP, C, G, S], F32, kind="ExternalInput")
    wbd = nc.dram_tensor("wbd", [128, 128], F32, kind="ExternalInput")
    ident = nc.dram_tensor("ident", [128, 128], F32, kind="ExternalInput")
    sel = nc.dram_tensor("sel", [32, 2, 128], F32, kind="ExternalInput")
    bb = nc.dram_tensor("b128", [128, 1], F32, kind="ExternalInput")
    ys = nc.dram_tensor("y", [BP, C, G, S], F32, kind="ExternalOutput")

    with tile.TileContext(nc) as tc, ExitStack() as ctx:
        consts = ctx.enter_context(tc.tile_pool(name="consts", bufs=1))
        xpool = ctx.enter_context(tc.tile_pool(name="xpool", bufs=3))
        ypool = ctx.enter_context(tc.tile_pool(name="ypool", bufs=n_t))
        opool = ctx.enter_context(tc.tile_pool(name="opool", bufs=4))
        spool = ctx.enter_context(tc.tile_pool(name="stats", bufs=2))
        small = ctx.enter_context(tc.tile_pool(name="small", bufs=2))
        psum = ctx.enter_context(tc.tile_pool(name="psum", bufs=3, space="PSUM"))
        psmall = ctx.enter_context(tc.tile_pool(name="psmall", bufs=2, space="PSUM"))

        wbd_sb = consts.tile([128, 128], F32)
        nc.gpsimd.dma_start(out=wbd_sb, in_=wbd[:])
        id_sb = consts.tile([128, 128], F32)
        nc.gpsimd.dma_start(out=id_sb, in_=ident[:])
        sel_sb = consts.tile([32, 2, 128], F32)
        nc.gpsimd.dma_start(out=sel_sb, in_=sel[:])
        b_sb = consts.tile([128, 1], F32)
        nc.gpsimd.dma_start(out=b_sb, in_=bb[:])

        state = {}

        def pass1_tile(bi, i):
            """DMA in x tile i of batch bi, conv+lrelu into resident y, stats."""
            st_min, st_max, y_tiles = state[bi][:3]
            xt = xpool.tile([128, ts], F32, tag="x")
            nc.sync.dma_start(out=xt, in_=xs[bi, :, :, i * ts:(i + 1) * ts])
            yt = ypool.tile([128, ts], F32, tag="y")
            for j in range(ts // PT):
                pt = psum.tile([128, PT], F32, tag="ps")
                for k in range(PT // MM):
                    c0 = k * MM
                    nc.tensor.matmul(
                        pt[:, c0:c0 + MM],
                        wbd_sb,
                        xt[:, j * PT + c0:j * PT + c0 + MM],
                        start=True,
                        stop=True,
                    )
                # y = leaky_relu(conv + b): fused PSUM->SBUF on ScalarE
                nc.scalar.activation(
                    out=yt[:, j * PT:(j + 1) * PT],
                    in_=pt,
                    func=AF.Prelu,
                    bias=b_sb,
                    scale=1.0,
                    alpha=NEG_SLOPE,
                )
            nc.vector.tensor_reduce(
                out=st_min[:, i:i + 1], in_=yt, axis=AX.X, op=ALU.min
            )
            nc.vector.tensor_reduce(
                out=st_max[:, i:i + 1], in_=yt, axis=AX.X, op=ALU.max
            )
            y_tiles.append(yt)

        def stats_fold(bi):
            """Fold per-partition stats into per-partition scale/bias [128,2]."""
            st_min, st_max = state[bi][:2]
            s2 = small.tile([128, 2], F32, tag="s2")
            nc.vector.tensor_reduce(out=s2[:, 0:1], in_=st_min, axis=AX.X, op=ALU.min)
            nc.vector.tensor_reduce(out=s2[:, 1:2], in_=st_max, axis=AX.X, op=ALU.max)
            # transpose [128,1] stats into free dim (partition 0)
            ptr_min = psmall.tile([1, 128], F32, tag="psm")
            nc.tensor.transpose(ptr_min, s2[:, 0:1], id_sb)
            ptr_max = psmall.tile([1, 128], F32, tag="psm")
            nc.tensor.transpose(ptr_max, s2[:, 1:2], id_sb)
            tl = small.tile([1, 256], F32, tag="tl")
            nc.scalar.copy(out=tl[:, 0:128], in_=ptr_min)
            nc.scalar.copy(out=tl[:, 128:256], in_=ptr_max)
            # reduce over the 8 groups (free index p = o*8+g)
            u = small.tile([1, 32], F32, tag="u")
            nc.vector.tensor_reduce(
                out=u[:, 0:16],
                in_=tl[:, 0:128].rearrange("p (o g) -> p o g", g=G),
                axis=AX.X,
                op=ALU.min,
            )
            nc.vector.tensor_reduce(
                out=u[:, 16:32],
                in_=tl[:, 128:256].rearrange("p (o g) -> p o g", g=G),
                axis=AX.X,
                op=ALU.max,
            )
            # scale = 1/(mx-mn+eps); nbias = -mn*scale
            v = small.tile([1, 16], F32, tag="v")
            nc.vector.tensor_sub(out=v, in0=u[:, 16:32], in1=u[:, 0:16])
            vv = small.tile([1, 16], F32, tag="vv")
            nc.vector.tensor_scalar(
                out=vv, in0=v, scalar1=EPS, scalar2=None, op0=ALU.add
            )
            pk = small.tile([1, 32], F32, tag="pk")
            nc.vector.reciprocal(out=pk[:, 0:16], in_=vv)
            tmp = small.tile([1, 16], F32, tag="tmp")
            nc.vector.tensor_mul(out=tmp, in0=u[:, 0:16], in1=pk[:, 0:16])
            nc.vector.tensor_scalar(
                out=pk[:, 16:32], in0=tmp, scalar1=-1.0, scalar2=None, op0=ALU.mult
            )
            # broadcast [1,32] free-dim -> per-partition [128,2] via transpose
            # + selector matmuls (sel[k,0,p]=d(k==p//8), sel[k,1,p]=d(k-16==p//8))
            pz = psmall.tile([32, 1], F32, tag="psm")
            nc.tensor.transpose(pz, pk, id_sb[0:1, 0:1])
            zs = small.tile([32, 1], F32, tag="zs")
            nc.scalar.copy(out=zs, in_=pz)
            pb1 = psmall.tile([128, 1], F32, tag="psm")
            nc.tensor.matmul(pb1, sel_sb[:, 0, :], zs, start=True, stop=True)
            pb2 = psmall.tile([128, 1], F32, tag="psm")
            nc.tensor.matmul(pb2, sel_sb[:, 1, :], zs, start=True, stop=True)
            sc = small.tile([128, 2], F32, tag="sc")
            nc.scalar.copy(out=sc[:, 0:1], in_=pb1)
            nc.scalar.copy(out=sc[:, 1:2], in_=pb2)
            return sc

        def pass2_tile(bi, i, sc):
            """Normalize resident y tile (GpSimd) and stream out."""
            y_tiles = state[bi][2]
            ot = opool.tile([128, ts], F32, tag="o")
            nc.gpsimd.tensor_scalar(
                out=ot,
                in0=y_tiles[i],
                scalar1=sc[:, 0:1],
                scalar2=sc[:, 1:2],
                op0=ALU.mult,
                op1=ALU.add,
            )
            nc.scalar.dma_start(out=ys[bi, :, :, i * ts:(i + 1) * ts], in_=ot)

        # software pipeline: pass1(0); then per batch: pre-emit the first
        # next-batch tiles (keeps PE busy through the stats fold), fold
        # stats, then interleave pass2(bi) with the rest of pass1(bi+1).
        PRE = 2  # spare ypool slots usable before pass2 frees more
        for bi in range(BP):
            state[bi] = (
                spool.tile([128, n_t], F32, tag="stmin", name=f"stmin{bi}"),
                spool.tile([128, n_t], F32, tag="stmax", name=f"stmax{bi}"),
                [],
                None,
            )
        for i in range(n_t):
            pass1_tile(0, i)
        for bi in range(BP):
            if bi + 1 < BP:
                for i in range(PRE):
                    pass1_tile(bi + 1, i)
            sc = stats_fold(bi)
            for i in range(n_t):
                pass2_tile(bi, i, sc)
                if bi + 1 < BP and i + PRE < n_t:
                    pass1_tile(bi + 1, i + PRE)

    nc.compile()
    return nc


def host_consts(w, b):
    """Host-side tiny constant tensors fed to every core."""
    w = np.asarray(w, np.float32).reshape(16)
    b = np.asarray(b, np.float32).reshape(1)
    W2 = np.stack([np.roll(w, o) for o in range(16)], axis=0)   # [O,C]
    wbd = np.kron(W2.T.copy(), np.eye(G, dtype=np.float32))     # [128,128]
    wbd = np.ascontiguousarray(wbd, np.float32)
    ident = np.eye(128, dtype=np.float32)
    sel = np.zeros((32, 2, 128), np.float32)
    for p in range(128):
        sel[p // G, 0, p] = 1.0
        sel[16 + p // G, 1, p] = 1.0
    b128 = np.full((128, 1), float(b[0]), np.float32)
    return wbd, ident, sel, b128


_NC = None
LAST_RESULTS = None


def kernel(x, w, b):
    global _NC, LAST_RESULTS
    x = np.ascontiguousarray(np.asarray(x, np.float32))
    assert x.shape == (B, C, H, W)
    if _NC is None:
        _NC = build_nc()
    wbd, ident, sel, b128 = host_consts(w, b)

    xg = x.reshape(N_CORES, BP, C, G, S_FULL)
    in_maps = [
        {
            "x": np.ascontiguousarray(xg[ci]),
            "wbd": wbd,
            "ident": ident,
            "sel": sel,
            "b128": b128,
        }
        for ci in range(N_CORES)
    ]
    res = run_bass_kernel_spmd(_NC, in_maps, core_ids=list(range(N_CORES)))
    LAST_RESULTS = res
    out = np.concatenate([r["y"].reshape(BP, C, H, W) for r in res.results], axis=0)
    return out
